# revision 6
# baseline (speedup 1.0000x reference)
"""Trainium2 Bass kernel for the MIP-GNN message-passing network.

Strategy (8 NeuronCores, SPMD + collectives):
  - Nodes dst-sharded by contiguous id range: core c owns nodes
    [12500c, 12500(c+1)), padded to MPAD=12800 local rows.
  - Per layer: each core computes y = x @ w_cons for its rows (transposed
    activation layout [64, cols]); four AllGathers distribute y sub-blocks
    (3200 local rows each) to per-group tables T_g [8*3200=25600, 64]
    (int16-indexable for dma_gather).
  - Edges are host-sorted by dst, split into 4 groups by the src's local
    sub-block; per (group, 512-dst-window) the slots are padded to whole
    128-slot chunks (chunk counts maxed over cores so one SPMD program fits
    all cores; per-core variation lives in the index/dstcol data).
  - dma_gather fetches message rows; a per-chunk segment matrix (built on
    device: iota + is_equal against host-provided dst columns, bf16)
    scatter-sums them into PSUM [64, 512] windows via matmuls.
    The x @ root seed matmul initializes the window; bias+ReLU via ACT.
  - Final head (fc1..fc4) computed on every core for its rows; host keeps
    cores 0-3 (the var nodes).
"""

import numpy as np

NCORES = 8
NV = 50000
NN = 100000
BLK = 12500
MPAD = 12800
SUB = 3200  # MPAD // 4, local rows per group sub-block
TROWS = NCORES * SUB  # 25600 rows per group table
P = 128
D = 64
W = 512  # dst window width == psum free width
NRANGE = MPAD // W  # 25
NG = 4
NIDX = 4096  # idx per gather instruction (multiple of 128)
NLAYER = 4

_CACHE = {}


def _wrap_idx_segment(flat):
    """Wrap a (n % 16 == 0) flat int16 idx list into the [128, n/16] SBUF
    layout dma_gather expects (idx i at partition i%16, col i//16, replicated
    8x across partition groups)."""
    n = flat.shape[0]
    blk = flat.reshape(n // 16, 16).T  # [16, n/16]
    return np.tile(blk, (8, 1)).astype(np.int16)


def _preprocess(edge_index):
    """Host-side sharding: per-core per-group gather indices, dst columns,
    and the global (core-independent) chunk schedule."""
    src = np.asarray(edge_index[0], dtype=np.int64)
    dst = np.asarray(edge_index[1], dtype=np.int64)

    owner = dst // BLK
    o_src = src // BLK
    l_src = src - o_src * BLK
    grp = l_src // SUB
    r16_all = o_src * SUB + (l_src - grp * SUB)  # row in T_g
    dloc_all = dst - owner * BLK

    # per (core, group, cell) slot lists
    per = {}
    n_ch = np.zeros((NCORES, NG, NRANGE), dtype=np.int64)
    for c in range(NCORES):
        selc = owner == c
        s_r16 = r16_all[selc]
        s_grp = grp[selc]
        s_dl = dloc_all[selc]
        for g in range(NG):
            selg = s_grp == g
            r16 = s_r16[selg]
            dl = s_dl[selg]
            order = np.argsort(dl, kind="stable")
            r16 = r16[order]
            dl = dl[order]
            cell = dl // W
            # cells are contiguous runs since dl sorted
            bounds = np.searchsorted(cell, np.arange(NRANGE + 1))
            for r in range(NRANGE):
                a, b = bounds[r], bounds[r + 1]
                per[(c, g, r)] = (r16[a:b], dl[a:b] - r * W)
                n_ch[c, g, r] = -(-(b - a) // P)

    n_ch_max = n_ch.max(axis=0)  # [NG, NRANGE]

    # build padded per-core arrays
    idx_arrays = []  # [core][g] -> wrapped int16 [128, TOT_g/16]
    col_arrays = []  # [core][g] -> f32 [128, TOTCH_g]
    for c in range(NCORES):
        idx_g, col_g = [], []
        for g in range(NG):
            parts_r16, parts_col = [], []
            for r in range(NRANGE):
                r16, col = per[(c, g, r)]
                tot = int(n_ch_max[g, r]) * P
                pr16 = np.zeros(tot, dtype=np.int64)
                pcol = np.full(tot, -1.0, dtype=np.float32)
                pr16[: len(r16)] = r16
                pcol[: len(col)] = col
                parts_r16.append(pr16)
                parts_col.append(pcol)
            gr16 = np.concatenate(parts_r16) if parts_r16 else np.zeros(0, np.int64)
            gcol = np.concatenate(parts_col) if parts_col else np.zeros(0, np.float32)
            idx_g.append(_wrap_idx_segment(gr16.astype(np.int16)))
            col_g.append(np.ascontiguousarray(gcol.reshape(-1, P).T.astype(np.float32)))
            assert gr16.max(initial=0) < TROWS
        idx_arrays.append(idx_g)
        col_arrays.append(col_g)

    # global schedule: per group, gather instruction sizes
    tot_slots = [int(n_ch_max[g].sum()) * P for g in range(NG)]
    instrs = []  # [g] -> list of (slot_offset, num_idxs)
    for g in range(NG):
        lst = []
        a = 0
        while a < tot_slots[g]:
            n = min(NIDX, tot_slots[g] - a)
            lst.append((a, n))
            a += n
        instrs.append(lst)
    return idx_arrays, col_arrays, n_ch_max, tot_slots, instrs


def _build_program(n_ch_max, tot_slots, instrs):
    import concourse.bass as bass
    import concourse.bacc as bacc
    import concourse.tile as tile
    import concourse.mybir as mybir

    f32 = mybir.dt.float32
    bf16 = mybir.dt.bfloat16
    i16 = mybir.dt.int16
    RELU = mybir.ActivationFunctionType.Relu

    nc = bacc.Bacc("TRN2", target_bir_lowering=False, debug=False,
                   num_devices=NCORES)

    # ---- I/O ----
    feat = nc.dram_tensor("feat", [MPAD // P, 2, P], f32, kind="ExternalInput")
    w1b = nc.dram_tensor("w1b", [2, D - 1], f32, kind="ExternalInput")
    w2b = nc.dram_tensor("w2b", [D - 1, D - 1], f32, kind="ExternalInput")
    b2e = nc.dram_tensor("b2e", [D - 1, 1], f32, kind="ExternalInput")
    cons_d, root_d, cbias_d = [], [], []
    for l in range(NLAYER):
        cons_d.append(nc.dram_tensor(f"cons{l}", [D, D], f32, kind="ExternalInput"))
        root_d.append(nc.dram_tensor(f"root{l}", [D, D], f32, kind="ExternalInput"))
        cbias_d.append(nc.dram_tensor(f"cbias{l}", [D, 1], f32, kind="ExternalInput"))
    fc1_d = nc.dram_tensor("fc1", [5 * D, D], f32, kind="ExternalInput")
    fc2_d = nc.dram_tensor("fc2", [D, D], f32, kind="ExternalInput")
    fc3_d = nc.dram_tensor("fc3", [D, D], f32, kind="ExternalInput")
    fc4_d = nc.dram_tensor("fc4", [D, 1], f32, kind="ExternalInput")
    fb1_d = nc.dram_tensor("fb1", [D, 1], f32, kind="ExternalInput")
    fb2_d = nc.dram_tensor("fb2", [D, 1], f32, kind="ExternalInput")
    fb3_d = nc.dram_tensor("fb3", [D, 1], f32, kind="ExternalInput")
    fb4_d = nc.dram_tensor("fb4", [1, 1], f32, kind="ExternalInput")
    idx_d = [nc.dram_tensor(f"idx{g}", [P, tot_slots[g] // 16], i16,
                            kind="ExternalInput") for g in range(NG)]
    col_d = [nc.dram_tensor(f"col{g}", [P, tot_slots[g] // P], f32,
                            kind="ExternalInput") for g in range(NG)]
    out_d = nc.dram_tensor("out", [1, MPAD], f32, kind="ExternalOutput")

    # ---- internal DRAM ----
    ybounce = [[nc.dram_tensor(f"yb{pp}_{g}", [SUB, D], f32, kind="Internal")
                for g in range(NG)] for pp in range(2)]
    tabs = [[nc.dram_tensor(f"tab{pp}_{g}", [TROWS, D], f32, kind="Internal",
                            addr_space="Shared") for g in range(NG)]
            for pp in range(2)]
    xsl = [nc.dram_tensor(f"xsl{l}", [D, MPAD], f32, kind="Internal")
           for l in range(NLAYER + 1)]

    with tile.TileContext(nc) as tc:
        with (
            tc.tile_pool(name="persist", bufs=1) as pers,
            tc.tile_pool(name="io", bufs=3) as io,
            tc.tile_pool(name="gth", bufs=3) as gth,
            tc.tile_pool(name="seg", bufs=3) as segp,
            tc.tile_pool(name="sml", bufs=3) as sml,
            tc.tile_pool(name="psA", bufs=4, space="PSUM") as psA,
            tc.tile_pool(name="psB", bufs=3, space="PSUM") as psB,
        ):
            # ---- persistent SBUF ----
            xT = pers.tile([D, MPAD], f32)           # current-layer output
            accT = pers.tile([D, MPAD], f32)         # window accumulators
            iota = pers.tile([P, W], f32)
            nc.gpsimd.iota(iota[:], pattern=[[1, W]], base=0,
                           channel_multiplier=0,
                           allow_small_or_imprecise_dtypes=True)
            w1b_t = pers.tile([2, D - 1], f32)
            nc.sync.dma_start(out=w1b_t[:], in_=w1b[:, :])
            w2b_t = pers.tile([D - 1, D - 1], f32)
            nc.sync.dma_start(out=w2b_t[:], in_=w2b[:, :])
            b2e_t = pers.tile([D - 1, 1], f32)
            nc.sync.dma_start(out=b2e_t[:], in_=b2e[:, :])
            cons_t, root_t, cbias_t = [], [], []
            for l in range(NLAYER):
                ct = pers.tile([D, D], f32, tag=f"cons{l}")
                nc.sync.dma_start(out=ct[:], in_=cons_d[l][:, :])
                cons_t.append(ct)
                rt = pers.tile([D, D], f32, tag=f"root{l}")
                nc.sync.dma_start(out=rt[:], in_=root_d[l][:, :])
                root_t.append(rt)
                bt = pers.tile([D, 1], f32, tag=f"cbias{l}")
                nc.sync.dma_start(out=bt[:], in_=cbias_d[l][:, :])
                cbias_t.append(bt)
            # fc1 is [320, 64]; SBUF partitions max 128, so load as 3 tiles
            fc1a = pers.tile([P, D], f32, tag="fc1a")
            nc.sync.dma_start(out=fc1a[:], in_=fc1_d[0:P, :])
            fc1b = pers.tile([P, D], f32, tag="fc1b")
            nc.sync.dma_start(out=fc1b[:], in_=fc1_d[P:2 * P, :])
            fc1c = pers.tile([D, D], f32, tag="fc1c")
            nc.sync.dma_start(out=fc1c[:], in_=fc1_d[2 * P:2 * P + D, :])
            fc2_t = pers.tile([D, D], f32, tag="fc2")
            nc.sync.dma_start(out=fc2_t[:], in_=fc2_d[:, :])
            fc3_t = pers.tile([D, D], f32, tag="fc3")
            nc.sync.dma_start(out=fc3_t[:], in_=fc3_d[:, :])
            fc4_t = pers.tile([D, 1], f32, tag="fc4")
            nc.sync.dma_start(out=fc4_t[:], in_=fc4_d[:, :])
            fb_t = []
            for nm, dd in (("fb1", fb1_d), ("fb2", fb2_d), ("fb3", fb3_d)):
                t = pers.tile([D, 1], f32, tag=nm)
                nc.sync.dma_start(out=t[:], in_=dd[:, :])
                fb_t.append(t)
            fb4_t = pers.tile([1, 1], f32, tag="fb4")
            nc.sync.dma_start(out=fb4_t[:], in_=fb4_d[:, :])

            def emit_y_tile(t, wcons, pp):
                """y rows [128t,128(t+1)) = (x @ wcons) from xT cols."""
                psy = psB.tile([P, D], f32, tag="small")
                nc.tensor.matmul(out=psy[:], lhsT=xT[:, t * P:(t + 1) * P],
                                 rhs=wcons[:], start=True, stop=True)
                ybuf = sml.tile([P, D], f32, tag="ybuf")
                nc.scalar.copy(out=ybuf[:], in_=psy[:])
                g = t // (SUB // P)
                row0 = (t % (SUB // P)) * P
                nc.sync.dma_start(out=ybounce[pp][g][row0:row0 + P, :], in_=ybuf[:])

            # ---- embed ----
            IDENT = mybir.ActivationFunctionType.Identity
            for t in range(MPAD // P):
                rhs2 = sml.tile([2, P], f32, tag="rhs2")
                nc.sync.dma_start(out=rhs2[:], in_=feat[t, :, :])
                ps1 = psB.tile([D - 1, P], f32, tag="small")
                nc.tensor.matmul(out=ps1[:], lhsT=w1b_t[:], rhs=rhs2[:],
                                 start=True, stop=True)
                h1 = sml.tile([D - 1, P], f32, tag="h1")
                nc.scalar.activation(out=h1[:], in_=ps1[:], func=RELU)
                ps2 = psB.tile([D - 1, P], f32, tag="small")
                nc.tensor.matmul(out=ps2[:], lhsT=w2b_t[:], rhs=h1[:],
                                 start=True, stop=True)
                nc.scalar.activation(out=xT[0:D - 1, t * P:(t + 1) * P],
                                     in_=ps2[:], func=IDENT, bias=b2e_t[:])
                nc.sync.dma_start(out=xT[D - 1:D, t * P:(t + 1) * P],
                                  in_=feat[t, 0:1, :])
            nc.sync.dma_start(out=xsl[0][:, :], in_=xT[:, :])
            for t in range(MPAD // P):
                emit_y_tile(t, cons_t[0], 0)

            # ---- layers ----
            for l in range(NLAYER):
                tab = tabs[l % 2]
                for g in range(NG):
                    nc.gpsimd.collective_compute(
                        "AllGather", mybir.AluOpType.bypass,
                        replica_groups=[list(range(NCORES))],
                        ins=[ybounce[l % 2][g][:, :]], outs=[tab[g][:, :]],
                    )
                for g in range(NG):
                    # lazy gather stream state
                    state = {"iptr": -1, "b0": 0, "nb": 0, "gbf": None}

                    def get_block(k, g=g, state=state):
                        """bf16 [128, 64] lhsT AP for global chunk k of group g."""
                        while k * P >= state["b0"] + state["nb"] * P or state["gbf"] is None:
                            state["iptr"] += 1
                            a, n = instrs[g][state["iptr"]]
                            it = io.tile([P, NIDX // 16], i16, tag="idx")
                            nc.sync.dma_start(
                                out=it[:, :n // 16],
                                in_=idx_d[g][:, a // 16:(a + n) // 16])
                            gt = gth.tile([P, (NIDX // P) * D], f32, tag="gf32")
                            nc.gpsimd.dma_gather(
                                out_ap=gt[:, :(n // P) * D].rearrange(
                                    "p (b d) -> p b d", d=D),
                                in_ap=tab[g][:, :],
                                idxs_ap=it[:, :n // 16],
                                num_idxs=n, num_idxs_reg=n, elem_size=D,
                                single_packet=False)
                            gbf = gth.tile([P, (NIDX // P) * D], bf16, tag="gbf")
                            nc.vector.tensor_copy(out=gbf[:, :(n // P) * D],
                                                  in_=gt[:, :(n // P) * D])
                            state["b0"], state["nb"] = a, n // P
                            state["gbf"] = gbf
                        b = k - state["b0"] // P
                        return state["gbf"][:, b * D:(b + 1) * D]

                    kglob = 0
                    for r in range(NRANGE):
                        nch = int(n_ch_max[g, r])
                        if g == 0:
                            seedx = sml.tile([D, W], f32, tag="seedx")
                            nc.sync.dma_start(out=seedx[:],
                                              in_=xsl[l][:, r * W:(r + 1) * W])
                        if nch == 0 and g != 0:
                            continue
                        ps = psA.tile([D, W], f32, tag="agg")
                        if g == 0:
                            nc.tensor.matmul(out=ps[:], lhsT=root_t[l][:],
                                             rhs=seedx[:], start=True,
                                             stop=(nch == 0))
                        if nch > 0:
                            dcol = sml.tile([P, max(nch, 1)], f32, tag="dcol")
                            nc.sync.dma_start(
                                out=dcol[:, :nch],
                                in_=col_d[g][:, kglob:kglob + nch])
                            for k in range(nch):
                                seg = segp.tile([P, W], bf16, tag="seg")
                                nc.vector.tensor_tensor(
                                    out=seg[:], in0=iota[:],
                                    in1=dcol[:, k:k + 1].to_broadcast([P, W]),
                                    op=mybir.AluOpType.is_equal)
                                nc.tensor.matmul(
                                    out=ps[:], lhsT=get_block(kglob + k),
                                    rhs=seg[:],
                                    start=(g > 0 and k == 0),
                                    stop=(k == nch - 1))
                            kglob += nch
                        # accumulate into accT
                        a0 = r * W
                        if g == 0:
                            nc.vector.tensor_copy(out=accT[:, a0:a0 + W], in_=ps[:])
                        else:
                            nc.vector.tensor_add(out=accT[:, a0:a0 + W],
                                                 in0=accT[:, a0:a0 + W], in1=ps[:])
                        if g == NG - 1:
                            nc.scalar.activation(out=xT[:, a0:a0 + W],
                                                 in_=accT[:, a0:a0 + W],
                                                 func=RELU, bias=cbias_t[l][:])
                            if l < NLAYER - 1:
                                for s in range(W // P):
                                    emit_y_tile(r * (W // P) + s,
                                                cons_t[l + 1], (l + 1) % 2)
                nc.sync.dma_start(out=xsl[l + 1][:, :], in_=xT[:, :])

            # ---- head ----
            for r in range(NRANGE):
                cs = slice(r * W, (r + 1) * W)
                hc0 = sml.tile([P, W], f32, tag="hc")
                nc.sync.dma_start(out=hc0[0:D, :], in_=xsl[0][:, cs])
                nc.sync.dma_start(out=hc0[D:2 * D, :], in_=xsl[1][:, cs])
                hc1 = sml.tile([P, W], f32, tag="hc2")
                nc.sync.dma_start(out=hc1[0:D, :], in_=xsl[2][:, cs])
                nc.sync.dma_start(out=hc1[D:2 * D, :], in_=xsl[3][:, cs])
                hc2 = sml.tile([D, W], f32, tag="hc3")
                nc.sync.dma_start(out=hc2[:], in_=xsl[4][:, cs])
                ph = psA.tile([D, W], f32, tag="agg")
                nc.tensor.matmul(out=ph[:], lhsT=fc1a[:], rhs=hc0[:],
                                 start=True, stop=False)
                nc.tensor.matmul(out=ph[:], lhsT=fc1b[:], rhs=hc1[:],
                                 start=False, stop=False)
                nc.tensor.matmul(out=ph[:], lhsT=fc1c[:], rhs=hc2[:],
                                 start=False, stop=True)
                h1t = sml.tile([D, W], f32, tag="h1t")
                nc.scalar.activation(out=h1t[:], in_=ph[:], func=RELU,
                                     bias=fb_t[0][:])
                ph2 = psA.tile([D, W], f32, tag="agg")
                nc.tensor.matmul(out=ph2[:], lhsT=fc2_t[:], rhs=h1t[:],
                                 start=True, stop=True)
                h2t = sml.tile([D, W], f32, tag="h1t")
                nc.scalar.activation(out=h2t[:], in_=ph2[:], func=RELU,
                                     bias=fb_t[1][:])
                ph3 = psA.tile([D, W], f32, tag="agg")
                nc.tensor.matmul(out=ph3[:], lhsT=fc3_t[:], rhs=h2t[:],
                                 start=True, stop=True)
                h3t = sml.tile([D, W], f32, tag="h1t")
                nc.scalar.activation(out=h3t[:], in_=ph3[:], func=RELU,
                                     bias=fb_t[2][:])
                ph4 = psB.tile([1, W], f32, tag="small")
                nc.tensor.matmul(out=ph4[:], lhsT=fc4_t[:], rhs=h3t[:],
                                 start=True, stop=True)
                ot = sml.tile([1, W], f32, tag="ot")
                nc.scalar.add(out=ot[:], in_=ph4[:], add=fb4_t[:])
                nc.sync.dma_start(out=out_d[:, cs], in_=ot[:])

    nc.compile()
    return nc


def kernel(var_node_features, con_node_features, node_types, assoc_var,
           assoc_con, edge_index, edge_types, edge_features, params):
    from concourse import bass_utils

    var_f = np.asarray(var_node_features, dtype=np.float32)
    con_f = np.asarray(con_node_features, dtype=np.float32)
    av = np.asarray(assoc_var, dtype=np.int64)
    ac = np.asarray(assoc_con, dtype=np.int64)
    assert np.array_equal(av, np.arange(NV)) and np.array_equal(
        ac, np.arange(NV, NN)), "kernel assumes arange assoc layout"
    p = {k: np.asarray(v, dtype=np.float32) for k, v in params.items()}

    ei = np.asarray(edge_index, dtype=np.int64)
    idx_arrays, col_arrays, n_ch_max, tot_slots, instrs = _preprocess(ei)

    key = ("prog", tuple(tot_slots), n_ch_max.tobytes())
    if key not in _CACHE:
        _CACHE.clear()
        _CACHE[key] = _build_program(n_ch_max, tot_slots, instrs)
    nc = _CACHE[key]

    def mlp_w(c):
        pre = "var" if c < 4 else "con"
        w1 = np.concatenate([p[f"{pre}_w1"], p[f"{pre}_b1"][None, :]], axis=0)
        w2 = p[f"{pre}_w2"]
        b2 = p[f"{pre}_b2"].reshape(D - 1, 1)
        return (w1.astype(np.float32), w2.astype(np.float32),
                b2.astype(np.float32))

    in_maps = []
    for c in range(NCORES):
        gid0 = c * BLK
        if c < 4:
            raw = var_f[gid0:gid0 + BLK, 0]
        else:
            raw = con_f[gid0 - NV:gid0 - NV + BLK, 0]
        feat = np.zeros(MPAD, dtype=np.float32)
        feat[:BLK] = raw
        featp = np.empty((MPAD // P, 2, P), dtype=np.float32)
        featp[:, 0, :] = feat.reshape(MPAD // P, P)
        featp[:, 1, :] = 1.0
        w1, w2, b2 = mlp_w(c)
        m = {
            "feat": featp,
            "w1b": w1, "w2b": w2, "b2e": b2,
            "fc1": p["fc1_w"], "fc2": p["fc2_w"], "fc3": p["fc3_w"],
            "fc4": p["fc4_w"],
            "fb1": p["fc1_b"].reshape(D, 1), "fb2": p["fc2_b"].reshape(D, 1),
            "fb3": p["fc3_b"].reshape(D, 1),
            "fb4": p["fc4_b"].reshape(1, 1),
        }
        for l in range(NLAYER):
            m[f"cons{l}"] = p[f"conv{l}_w_cons"]
            m[f"root{l}"] = p[f"conv{l}_root"]
            m[f"cbias{l}"] = p[f"conv{l}_bias"].reshape(D, 1)
        for g in range(NG):
            m[f"idx{g}"] = idx_arrays[c][g]
            m[f"col{g}"] = col_arrays[c][g]
        in_maps.append(m)

    res = bass_utils.run_bass_kernel_spmd(nc, in_maps,
                                          core_ids=list(range(NCORES)))
    out = np.concatenate([res.results[c]["out"][0, :BLK] for c in range(4)])
    return out.astype(np.float32)
